# revision 26
# baseline (speedup 1.0000x reference)
"""MoE routing kernel for Trainium2 (8 NeuronCores).

Math (per reference):
  S = sigmoid(x @ Wg^T); top-2 gates G at indices I; w[t,e] = G if selected else 0
  down = sum_e w[:,e] * (x @ Wd[e]^T)          # [T, Dg]  (shared across experts)
  up   = sum_e w[:,e] * (down @ Wu[e]^T)       # [T, D]

Strategy: data-parallel over tokens — each of the 8 cores handles T/8 = 512
tokens and computes all 8 experts densely (top-2 applied via gate weights).
Host passes x^T / Wg^T / per-expert-transposed Wd, Wu so every on-chip matmul
has its contraction dim on partitions with zero large on-chip transposes.

Per-core dataflow (tokens token-major, 4 tiles of 128):
  gate:  ST[e,t] = Wg^T-chunks (lhsT) x xT-chunks (rhs, fp32 — bf16 scores
         would flip ~1.4% of top-2 selections and blow the error budget)
         accumulated in PSUM; PE-transpose 128-token slices; top-2 via two
         reduce_max passes; w = sigmoid(Z) * (Z >= second_max)
  down:  P_pair[t, 2*Dg] += xT-chunk (lhsT, bf16) x WdT-pair-chunk (rhs) over
         16 K-chunks; Wd pairs stream through 2 SBUF buffers (each pair dies
         after its matmuls); combine dacc = sum_e w_e * P_e on DVE
  z:     Z_e = w_e * dacc (ACT per-partition scale, bf16); PE-transpose to
         ZT_e[g, t]
  up:    U[t, dblk] += ZT_e-chunk (lhsT) x WuT-chunk (rhs) accumulated over
         (e, g-chunk) in PSUM; assemble bf16 rows; one 512 KiB DMA per token
         tile (out is bf16; host casts back to f32).
"""

import numpy as np
import ml_dtypes

import concourse.bass as bass
import concourse.mybir as mybir
import concourse.tile as tile
from concourse.bacc import Bacc
from concourse.bass_utils import run_bass_kernel_spmd

BF16 = mybir.dt.bfloat16
F32 = mybir.dt.float32
I32 = mybir.dt.int32
AF = mybir.ActivationFunctionType
ALU = mybir.AluOpType
AX = mybir.AxisListType

NCORES = 8
B, L, D, E, DG = 2, 2048, 2048, 8, 256
T = B * L            # 4096 tokens
TC = T // NCORES     # 512 tokens per core
P = 128
NDC = D // P         # 16 contraction chunks over D
NTT = TC // P        # 4 token tiles per core
DBLK = 512           # free-dim block for the up matmul
NDB = D // DBLK      # 4
NPAIR = E // 2       # 4 expert pairs (2 experts share one PSUM bank)
NGC = DG // P        # 2 contraction chunks over Dg



def build_moe(nc: bass.Bass, repeat: int = 1):
    # All inputs are host-prepacked partition-major so every DMA row is one
    # long contiguous run (few descriptors per transfer).
    xhi = nc.dram_tensor("xhi", [P, NDC, TC], BF16, kind="ExternalInput")
    xlo = nc.dram_tensor("xlo", [P, NDC, TC], BF16, kind="ExternalInput")
    Wghl = nc.dram_tensor("Wghl", [P, 2, NDC, E], BF16, kind="ExternalInput")
    Wdp = nc.dram_tensor("Wdp", [NPAIR, P, NDC, 2 * DG], BF16, kind="ExternalInput")
    WuTt = nc.dram_tensor("WuTt", [E, P, NGC, D], BF16, kind="ExternalInput")
    idb = nc.dram_tensor("idb", [P, P], BF16, kind="ExternalInput")
    idf = nc.dram_tensor("idf", [P, P], F32, kind="ExternalInput")
    out = nc.dram_tensor("out", [TC, D], BF16, kind="ExternalOutput")

    with tile.TileContext(nc) as tc:
        with (
            tc.tile_pool(name="res", bufs=1) as res,
            tc.tile_pool(name="stream", bufs=3) as stream,
            tc.tile_pool(name="small", bufs=2) as small,
            tc.tile_pool(name="ps", bufs=1, space="PSUM") as ps,
        ):
          # repeat>1 builds a timing NEFF that executes the whole kernel R
          # times back-to-back so fixed dispatch overhead cancels in
          # (t_R - t_1) / (R - 1).
          # constants load once; the repeat loop (timing NEFF) reuses them,
          # matching a single kernel() call where they also load once.
          ident_b = res.tile([P, P], BF16, tag="identb", name="ident_b")
          nc.sync.dma_start(ident_b[:], idb[:, :])
          ident_f = res.tile([E, E], F32, tag="identf", name="ident_f")
          nc.sync.dma_start(ident_f[:], idf[:E, :E])
          wg_sb = res.tile([P, 2, NDC, E], BF16, tag="wg", name="wg_sb")
          nc.sync.dma_start(wg_sb[:], Wghl[:, :, :, :])

          # PE warmup (first rep only): trip the HAM activity window so the
          # gate matmuls run at 2.4 GHz; later reps stay warm back-to-back.
          wps = ps.tile([P, P], F32, tag="bank", bufs=6, name="warm_ps")
          for _w in range(24):
              nc.tensor.matmul(wps[:], ident_b[:], ident_b[:], start=True, stop=True)

          for _rep in range(repeat):
            # x arrives pre-split as bf16 hi+lo (x = hi + lo exactly to
            # fp32-class precision). down consumes hi directly (no casts);
            # the gate accumulates hi*Wg_hi + hi*Wg_lo + lo*Wg_hi in fp32
            # PSUM -- dropped lo*lo term is ~4e-6 relative, selection-safe.
            xbf = res.tile([P, NDC, TC], BF16, tag="xbf", name="xbf")
            nc.sync.dma_start(xbf[:], xhi[:, :, :])
            xlo_sb = res.tile([P, NDC, TC], BF16, tag="xlo", name="xlo_sb")
            nc.sync.dma_start(xlo_sb[:], xlo[:, :, :])

            # gate matmuls are M=8 (8 experts): pack 4 dc-chunks into the 4
            # column-groups of the PE array (tile_position) so they run
            # concurrently; each 32-strip accumulates 12 products.
            st_ps = ps.tile([P, TC], F32, tag="bank", bufs=6, name="st_ps")
            XCH = 4          # dc-chunks per concurrent strip group
            NXC = NDC // XCH
            for xc in range(NXC):
                for sub in range(XCH):
                    dc = xc * XCH + sub
                    for term, (wh, xs) in enumerate(
                        ((0, xbf), (1, xbf), (0, xlo_sb))
                    ):
                        nc.tensor.matmul(
                            st_ps[32 * sub : 32 * sub + E, :],
                            wg_sb[:, wh, dc, :],
                            xs[:, dc, :],
                            start=(xc == 0 and term == 0),
                            stop=(xc == NXC - 1 and term == 2),
                            tile_position=(0, 32 * sub),
                        )

            # ---------- expert weight loads (overlap with compute) ----------
            # wd pairs stream through 2 buffers (each dies after its down
            # pair); wu stays resident (the sparse up phase consumes it
            # faster than it could stream)
            def load_wd(pr):
                t = stream.tile([P, NDC, 2 * DG], BF16, tag="wd", bufs=2, name=f"wd{pr}")
                nc.sync.dma_start(t[:], Wdp[pr])
                return t
            wd_sb = [load_wd(0), load_wd(1)]
            wu_sb = []
            for e in range(E):
                t = res.tile([P, NGC, D], BF16, tag=f"wu{e}", name=f"wu{e}")
                nc.sync.dma_start(t[:], WuTt[e])
                wu_sb.append(t)

            # ---------- gate: transpose to token-major, top-2, weights ----------
            st_sb = res.tile([E, TC], F32, tag="stsb", name="st_sb")
            nc.vector.tensor_copy(st_sb[:], st_ps[0:E, :])
            for j in range(1, XCH):
                nc.vector.tensor_tensor(
                    st_sb[:], st_sb[:], st_ps[32 * j : 32 * j + E, :], ALU.add
                )
            w_tiles = []
            for tt in range(NTT):
                ztok = ps.tile([P, E], F32, tag="tr", bufs=2, name=f"ztok{tt}")
                nc.tensor.transpose(
                    ztok[:], st_sb[:, tt * P : (tt + 1) * P], ident_f[:]
                )
                m1 = small.tile([P, 1], F32, tag="m1", name=f"m1_{tt}")
                nc.vector.reduce_max(m1[:], ztok[:], axis=AX.X)
                # tmp = Z + (Z == m1) * -1e30  (mask out the max)
                tmp = small.tile([P, E], F32, tag="tmp", name=f"tmp{tt}")
                nc.vector.tensor_scalar(
                    tmp[:], ztok[:], m1[:], -1e30, ALU.is_equal, ALU.mult
                )
                nc.vector.tensor_tensor(tmp[:], tmp[:], ztok[:], ALU.add)
                m2 = small.tile([P, 1], F32, tag="m2", name=f"m2_{tt}")
                nc.vector.reduce_max(m2[:], tmp[:], axis=AX.X)
                g = small.tile([P, E], F32, tag="g", name=f"g{tt}")
                nc.scalar.activation(g[:], ztok[:], AF.Sigmoid)
                msk = small.tile([P, E], F32, tag="msk", name=f"msk{tt}")
                nc.vector.tensor_scalar(msk[:], ztok[:], m2[:], None, ALU.is_ge)
                w = res.tile([P, E], F32, tag=f"w{tt}", name=f"w{tt}")
                nc.vector.tensor_tensor(w[:], g[:], msk[:], ALU.mult)
                w_tiles.append(w)

            # ---------- down (dense) ----------
            dacc_bf = res.tile([P, NTT, DG], BF16, tag="daccbf", name="dacc_bf")
            daccs = {}
            for pr in range(NPAIR):
                e0, e1 = 2 * pr, 2 * pr + 1
                if pr + 2 < NPAIR:
                    wd_sb.append(load_wd(pr + 2))
                for tt in range(NTT):
                    ts_ = slice(tt * P, (tt + 1) * P)
                    pt = ps.tile(
                        [P, 2 * DG], F32, tag="bank", bufs=6, name=f"pd{pr}_{tt}"
                    )
                    for dc in range(NDC):
                        nc.tensor.matmul(
                            pt[:],
                            xbf[:, dc, ts_],
                            wd_sb[pr][:, dc, :],
                            start=(dc == 0),
                            stop=(dc == NDC - 1),
                        )
                    if pr == 0:
                        dacc = stream.tile(
                            [P, DG], F32, tag="dacc", bufs=4, name=f"dacc{tt}"
                        )
                        daccs[tt] = dacc
                        nc.vector.tensor_scalar(
                            dacc[:], pt[:, 0:DG],
                            w_tiles[tt][:, e0 : e0 + 1], None, ALU.mult,
                        )
                    else:
                        dacc = daccs[tt]
                        nc.vector.scalar_tensor_tensor(
                            dacc[:], pt[:, 0:DG],
                            w_tiles[tt][:, e0 : e0 + 1], dacc[:],
                            ALU.mult, ALU.add,
                        )
                    nc.vector.scalar_tensor_tensor(
                        dacc_bf[:, tt, :] if pr == NPAIR - 1 else dacc[:],
                        pt[:, DG : 2 * DG],
                        w_tiles[tt][:, e1 : e1 + 1], dacc[:],
                        ALU.mult, ALU.add,
                    )

            # ---------- z^T via diagonal-weight matmuls ----------
            # zT_e[g, tok] = sum_k dacc_bf[k, g] * (w_e[k] * I[k, tok])
            # One N=512 matmul covers 4 experts (diags side by side); no PE
            # transposes, no ACT scales. All 16 zt matmuls run before the up
            # streams so the PE sees a single zt->up handoff bubble.
            zt_sb = res.tile([P, NGC, E, TC], BF16, tag="zt", name="zt_sb")
            for tt in range(NTT):
                ts_ = slice(tt * P, (tt + 1) * P)
                dg4s = []
                for eg in range(2):
                    dg4 = stream.tile([P, 4 * P], BF16, tag="diag", bufs=4, name=f"dg{tt}_{eg}")
                    for i in range(4):
                        e = 4 * eg + i
                        nc.vector.tensor_scalar(
                            dg4[:, i * P : (i + 1) * P], ident_b[:],
                            w_tiles[tt][:, e : e + 1], None, ALU.mult,
                        )
                    dg4s.append(dg4)
                for gc in range(NGC):
                    for eg in range(2):
                        ztp = ps.tile([P, 4 * P], F32, tag="bank", bufs=6, name=f"zt{tt}_{gc}_{eg}")
                        nc.tensor.matmul(
                            ztp[:],
                            dacc_bf[:, tt, gc * P : (gc + 1) * P],
                            dg4s[eg][:],
                            start=True,
                            stop=True,
                        )
                        nc.vector.tensor_copy(
                            zt_sb[:, gc, 4 * eg : 4 * eg + 4, ts_], ztp[:]
                        )

            # ---------- up ----------
            for tt in range(NTT):
                ts_ = slice(tt * P, (tt + 1) * P)
                usb = stream.tile([P, D], BF16, tag="usb", bufs=2, name=f"usb{tt}")
                for db in range(NDB):
                    u = ps.tile([P, DBLK], F32, tag="bank", bufs=6, name=f"u{tt}_{db}")
                    kk = 0
                    for e in range(E):
                        for gc in range(NGC):
                            nc.tensor.matmul(
                                u[:],
                                zt_sb[:, gc, e, ts_],
                                wu_sb[e][:, gc, db * DBLK : (db + 1) * DBLK],
                                start=(kk == 0),
                                stop=(kk == E * NGC - 1),
                            )
                            kk += 1
                    nc.scalar.copy(usb[:, db * DBLK : (db + 1) * DBLK], u[:])
                nc.sync.dma_start(out[tt * P : (tt + 1) * P, :], usb[:])
    return nc


_CACHE = {}


def get_nc(repeat: int = 1) -> bass.Bass:
    key = ("nc", repeat)
    if key not in _CACHE:
        nc = Bacc()
        build_moe(nc, repeat=repeat)
        nc.compile()
        _CACHE[key] = nc
    return _CACHE[key]


def _pmajor(a2d, pdim_chunks):
    """[D_outer*P, X] -> [P, D_outer, X] partition-major contiguous."""
    d, x = a2d.shape
    return np.ascontiguousarray(
        a2d.reshape(pdim_chunks, P, x).transpose(1, 0, 2)
    )


def prep_in_maps(x, Wg, Wd, Wu):
    bf = ml_dtypes.bfloat16
    xf = np.asarray(x, np.float32).reshape(T, D)
    xTf = np.ascontiguousarray(xf.T)                       # [D, T]
    WgTf = _pmajor(
        np.ascontiguousarray(np.asarray(Wg, np.float32).T), NDC
    )                                                      # [P, NDC, E] f32
    Wg_hi = WgTf.astype(bf)
    Wg_lo = (WgTf - Wg_hi.astype(np.float32)).astype(bf)
    Wghl_h = np.ascontiguousarray(np.stack([Wg_hi, Wg_lo], axis=1))
    WdT = np.asarray(Wd, np.float32).transpose(0, 2, 1)    # [E, D, DG]
    # pair p holds experts (2p, 2p+1) side by side on the free dim
    Wdp_c = np.concatenate([WdT[0::2], WdT[1::2]], axis=2).astype(bf)
    Wdp_h = np.ascontiguousarray(
        Wdp_c.reshape(NPAIR, NDC, P, 2 * DG).transpose(0, 2, 1, 3)
    )                                                      # [NPAIR, P, NDC, 2*DG]
    WuT_c = np.asarray(Wu, np.float32).transpose(0, 2, 1).astype(bf)  # [E, DG, D]
    WuT_h = np.ascontiguousarray(
        WuT_c.reshape(E, NGC, P, D).transpose(0, 2, 1, 3)
    )                                                      # [E, P, NGC, D]
    idb_h = np.eye(P, dtype=bf)
    idf_h = np.eye(P, dtype=np.float32)
    shared = dict(Wghl=Wghl_h, Wdp=Wdp_h, WuTt=WuT_h, idb=idb_h, idf=idf_h)
    in_maps = []
    for c in range(NCORES):
        m = dict(shared)
        xTc = _pmajor(
            np.ascontiguousarray(xTf[:, c * TC : (c + 1) * TC]), NDC
        )                                                  # [P, NDC, TC] f32
        x_hi = xTc.astype(bf)
        m["xhi"] = x_hi
        m["xlo"] = (xTc - x_hi.astype(np.float32)).astype(bf)
        in_maps.append(m)
    return in_maps


def kernel(x, Wg, Wd, Wu, k):
    assert int(k) == 2, f"kernel hardcodes top-2 routing, got k={k}"
    nc = get_nc()
    in_maps = prep_in_maps(x, Wg, Wd, Wu)
    res = run_bass_kernel_spmd(nc, in_maps, core_ids=list(range(NCORES)))
    outs = [np.asarray(res.results[c]["out"], dtype=np.float32) for c in range(NCORES)]
    return np.ascontiguousarray(
        np.concatenate(outs, axis=0).reshape(B, L, D), dtype=np.float32
    )


# revision 28
# speedup vs baseline: 1.0185x; 1.0185x over previous
"""MoE routing kernel for Trainium2 (8 NeuronCores).

Math (per reference):
  S = sigmoid(x @ Wg^T); top-2 gates G at indices I; w[t,e] = G if selected else 0
  down = sum_e w[:,e] * (x @ Wd[e]^T)          # [T, Dg]  (shared across experts)
  up   = sum_e w[:,e] * (down @ Wu[e]^T)       # [T, D]

Strategy: data-parallel over tokens — each of the 8 cores handles T/8 = 512
tokens and computes all 8 experts densely (top-2 applied via gate weights).
Host passes x^T / Wg^T / per-expert-transposed Wd, Wu so every on-chip matmul
has its contraction dim on partitions with zero large on-chip transposes.

Per-core dataflow (tokens token-major, 4 tiles of 128):
  gate:  ST[e,t] = Wg^T-chunks (lhsT) x xT-chunks (rhs, fp32 — bf16 scores
         would flip ~1.4% of top-2 selections and blow the error budget)
         accumulated in PSUM; PE-transpose 128-token slices; top-2 via two
         reduce_max passes; w = sigmoid(Z) * (Z >= second_max)
  down:  P_pair[t, 2*Dg] += xT-chunk (lhsT, bf16) x WdT-pair-chunk (rhs) over
         16 K-chunks; Wd pairs stream through 2 SBUF buffers (each pair dies
         after its matmuls); combine dacc = sum_e w_e * P_e on DVE
  z:     Z_e = w_e * dacc (ACT per-partition scale, bf16); PE-transpose to
         ZT_e[g, t]
  up:    U[t, dblk] += ZT_e-chunk (lhsT) x WuT-chunk (rhs) accumulated over
         (e, g-chunk) in PSUM; assemble bf16 rows; one 512 KiB DMA per token
         tile (out is bf16; host casts back to f32).
"""

import numpy as np
import ml_dtypes

import concourse.bass as bass
import concourse.mybir as mybir
import concourse.tile as tile
from concourse.bacc import Bacc
from concourse.bass_utils import run_bass_kernel_spmd

BF16 = mybir.dt.bfloat16
F32 = mybir.dt.float32
I32 = mybir.dt.int32
AF = mybir.ActivationFunctionType
ALU = mybir.AluOpType
AX = mybir.AxisListType

NCORES = 8
B, L, D, E, DG = 2, 2048, 2048, 8, 256
T = B * L            # 4096 tokens
TC = T // NCORES     # 512 tokens per core
P = 128
NDC = D // P         # 16 contraction chunks over D
NTT = TC // P        # 4 token tiles per core
DBLK = 512           # free-dim block for the up matmul
NDB = D // DBLK      # 4
NPAIR = E // 2       # 4 expert pairs (2 experts share one PSUM bank)
NGC = DG // P        # 2 contraction chunks over Dg



def build_moe(nc: bass.Bass, repeat: int = 1):
    # All inputs are host-prepacked partition-major so every DMA row is one
    # long contiguous run (few descriptors per transfer).
    xT = nc.dram_tensor("xT", [P, NDC, TC], F32, kind="ExternalInput")
    WgT = nc.dram_tensor("WgT", [P, NDC, E], F32, kind="ExternalInput")
    Wdp = nc.dram_tensor("Wdp", [NPAIR, P, NDC, 2 * DG], BF16, kind="ExternalInput")
    WuTt = nc.dram_tensor("WuTt", [E, P, NGC, D], BF16, kind="ExternalInput")
    idb = nc.dram_tensor("idb", [P, P], BF16, kind="ExternalInput")
    idf = nc.dram_tensor("idf", [P, P], F32, kind="ExternalInput")
    out = nc.dram_tensor("out", [TC, D], BF16, kind="ExternalOutput")

    with tile.TileContext(nc) as tc:
        with (
            tc.tile_pool(name="res", bufs=1) as res,
            tc.tile_pool(name="stream", bufs=3) as stream,
            tc.tile_pool(name="small", bufs=2) as small,
            tc.tile_pool(name="ps", bufs=1, space="PSUM") as ps,
        ):
          # repeat>1 builds a timing NEFF that executes the whole kernel R
          # times back-to-back so fixed dispatch overhead cancels in
          # (t_R - t_1) / (R - 1).
          # constants load once; the repeat loop (timing NEFF) reuses them,
          # matching a single kernel() call where they also load once.
          ident_b = res.tile([P, P], BF16, tag="identb", name="ident_b")
          nc.sync.dma_start(ident_b[:], idb[:, :])
          ident_f = res.tile([E, E], F32, tag="identf", name="ident_f")
          nc.sync.dma_start(ident_f[:], idf[:E, :E])
          wg_sb = res.tile([P, NDC, E], F32, tag="wg", name="wg_sb")
          nc.sync.dma_start(wg_sb[:], WgT[:, :, :])

          # PE warmup (first rep only): trip the HAM activity window so the
          # gate matmuls run at 2.4 GHz; later reps stay warm back-to-back.
          wps = ps.tile([P, P], F32, tag="bank", bufs=6, name="warm_ps")
          for _w in range(24):
              nc.tensor.matmul(wps[:], ident_b[:], ident_b[:], start=True, stop=True)

          for _rep in range(repeat):
            xbf = res.tile([P, NDC, TC], BF16, tag="xbf", name="xbf")

            # ---------- stream x^T in 4 big chunks: cast to bf16 + gate matmul ----------
            # gate matmuls are M=8 (8 experts): pack 4 dc-chunks into the 4
            # column-groups of the PE array (tile_position) so they run
            # concurrently; each 32-strip accumulates 4 of the 16 chunks.
            st_ps = ps.tile([P, TC], F32, tag="bank", bufs=6, name="st_ps")
            XCH = 4          # dc-chunks per DMA = one concurrent group
            NXC = NDC // XCH
            for xc in range(NXC):
                xt = stream.tile([P, XCH, TC], F32, tag="xt", bufs=4, name=f"xt{xc}")
                nc.sync.dma_start(xt[:], xT[:, xc * XCH : (xc + 1) * XCH, :])
                for sub in range(XCH):
                    nc.scalar.copy(
                        xbf[:, xc * XCH + sub, :], xt[:, sub, :]
                    )
                for sub in range(XCH):
                    dc = xc * XCH + sub
                    nc.tensor.matmul(
                        st_ps[32 * sub : 32 * sub + E, :],
                        wg_sb[:, dc, :],
                        xt[:, sub, :],
                        start=(xc == 0),
                        stop=(xc == NXC - 1),
                        tile_position=(0, 32 * sub),
                    )

            # ---------- expert weight loads (overlap with compute) ----------
            # wd pairs stream through 2 buffers (each dies after its down
            # pair); wu stays resident (the sparse up phase consumes it
            # faster than it could stream)
            def load_wd(pr):
                t = stream.tile([P, NDC, 2 * DG], BF16, tag="wd", bufs=3, name=f"wd{pr}")
                nc.sync.dma_start(t[:], Wdp[pr])
                return t
            wd_sb = [load_wd(0), load_wd(1)]
            wu_sb = []
            for e in range(E):
                t = res.tile([P, NGC, D], BF16, tag=f"wu{e}", name=f"wu{e}")
                nc.sync.dma_start(t[:], WuTt[e])
                wu_sb.append(t)

            # ---------- gate: transpose to token-major, top-2, weights ----------
            st_sb = res.tile([E, TC], F32, tag="stsb", name="st_sb")
            nc.vector.tensor_copy(st_sb[:], st_ps[0:E, :])
            for j in range(1, XCH):
                nc.vector.tensor_tensor(
                    st_sb[:], st_sb[:], st_ps[32 * j : 32 * j + E, :], ALU.add
                )
            w_tiles = []
            for tt in range(NTT):
                ztok = ps.tile([P, E], F32, tag="tr", bufs=2, name=f"ztok{tt}")
                nc.tensor.transpose(
                    ztok[:], st_sb[:, tt * P : (tt + 1) * P], ident_f[:]
                )
                m1 = small.tile([P, 1], F32, tag="m1", name=f"m1_{tt}")
                nc.vector.reduce_max(m1[:], ztok[:], axis=AX.X)
                # tmp = Z + (Z == m1) * -1e30  (mask out the max)
                tmp = small.tile([P, E], F32, tag="tmp", name=f"tmp{tt}")
                nc.vector.tensor_scalar(
                    tmp[:], ztok[:], m1[:], -1e30, ALU.is_equal, ALU.mult
                )
                nc.vector.tensor_tensor(tmp[:], tmp[:], ztok[:], ALU.add)
                m2 = small.tile([P, 1], F32, tag="m2", name=f"m2_{tt}")
                nc.vector.reduce_max(m2[:], tmp[:], axis=AX.X)
                g = small.tile([P, E], F32, tag="g", name=f"g{tt}")
                nc.scalar.activation(g[:], ztok[:], AF.Sigmoid)
                msk = small.tile([P, E], F32, tag="msk", name=f"msk{tt}")
                nc.vector.tensor_scalar(msk[:], ztok[:], m2[:], None, ALU.is_ge)
                w = res.tile([P, E], F32, tag=f"w{tt}", name=f"w{tt}")
                nc.vector.tensor_tensor(w[:], g[:], msk[:], ALU.mult)
                w_tiles.append(w)

            # diag tiles for the z^T matmuls: built on DVE right after the
            # gate so they never queue behind the down combines
            dg4s_all = []
            for tt in range(NTT):
                dg4s = []
                for eg in range(2):
                    dg4 = stream.tile([P, 4 * P], BF16, tag="diag", bufs=8, name=f"dg{tt}_{eg}")
                    for i in range(4):
                        e = 4 * eg + i
                        nc.vector.tensor_scalar(
                            dg4[:, i * P : (i + 1) * P], ident_b[:],
                            w_tiles[tt][:, e : e + 1], None, ALU.mult,
                        )
                    dg4s.append(dg4)
                dg4s_all.append(dg4s)

            # ---------- down (dense) ----------
            dacc_bf = res.tile([P, NTT, DG], BF16, tag="daccbf", name="dacc_bf")
            daccs = {}
            for pr in range(NPAIR):
                e0, e1 = 2 * pr, 2 * pr + 1
                if pr + 2 < NPAIR:
                    wd_sb.append(load_wd(pr + 2))
                for tt in range(NTT):
                    ts_ = slice(tt * P, (tt + 1) * P)
                    pt = ps.tile(
                        [P, 2 * DG], F32, tag="bank", bufs=6, name=f"pd{pr}_{tt}"
                    )
                    for dc in range(NDC):
                        nc.tensor.matmul(
                            pt[:],
                            xbf[:, dc, ts_],
                            wd_sb[pr][:, dc, :],
                            start=(dc == 0),
                            stop=(dc == NDC - 1),
                        )
                    if pr == 0:
                        dacc = stream.tile(
                            [P, DG], F32, tag="dacc", bufs=4, name=f"dacc{tt}"
                        )
                        daccs[tt] = dacc
                        nc.vector.tensor_scalar(
                            dacc[:], pt[:, 0:DG],
                            w_tiles[tt][:, e0 : e0 + 1], None, ALU.mult,
                        )
                    else:
                        dacc = daccs[tt]
                        nc.vector.scalar_tensor_tensor(
                            dacc[:], pt[:, 0:DG],
                            w_tiles[tt][:, e0 : e0 + 1], dacc[:],
                            ALU.mult, ALU.add,
                        )
                    nc.vector.scalar_tensor_tensor(
                        dacc_bf[:, tt, :] if pr == NPAIR - 1 else dacc[:],
                        pt[:, DG : 2 * DG],
                        w_tiles[tt][:, e1 : e1 + 1], dacc[:],
                        ALU.mult, ALU.add,
                    )

            # ---------- z^T via diagonal-weight matmuls ----------
            # zT_e[g, tok] = sum_k dacc_bf[k, g] * (w_e[k] * I[k, tok])
            # One N=512 matmul covers 4 experts (diags side by side); no PE
            # transposes, no ACT scales. All 16 zt matmuls run before the up
            # streams so the PE sees a single zt->up handoff bubble.
            zt_sb = res.tile([P, NGC, E, TC], BF16, tag="zt", name="zt_sb")
            for tt in range(NTT):
                ts_ = slice(tt * P, (tt + 1) * P)
                for gc in range(NGC):
                    for eg in range(2):
                        ztp = ps.tile([P, 4 * P], F32, tag="bank", bufs=6, name=f"zt{tt}_{gc}_{eg}")
                        nc.tensor.matmul(
                            ztp[:],
                            dacc_bf[:, tt, gc * P : (gc + 1) * P],
                            dg4s_all[tt][eg][:],
                            start=True,
                            stop=True,
                        )
                        nc.scalar.copy(
                            zt_sb[:, gc, 4 * eg : 4 * eg + 4, ts_], ztp[:]
                        )

            # ---------- up ----------
            for tt in range(NTT):
                ts_ = slice(tt * P, (tt + 1) * P)
                usb = stream.tile([P, D], BF16, tag="usb", bufs=2, name=f"usb{tt}")
                for db in range(NDB):
                    u = ps.tile([P, DBLK], F32, tag="bank", bufs=6, name=f"u{tt}_{db}")
                    kk = 0
                    for e in range(E):
                        for gc in range(NGC):
                            nc.tensor.matmul(
                                u[:],
                                zt_sb[:, gc, e, ts_],
                                wu_sb[e][:, gc, db * DBLK : (db + 1) * DBLK],
                                start=(kk == 0),
                                stop=(kk == E * NGC - 1),
                            )
                            kk += 1
                    nc.scalar.copy(usb[:, db * DBLK : (db + 1) * DBLK], u[:])
                nc.sync.dma_start(out[tt * P : (tt + 1) * P, :], usb[:])
    return nc


_CACHE = {}


def get_nc(repeat: int = 1) -> bass.Bass:
    key = ("nc", repeat)
    if key not in _CACHE:
        nc = Bacc()
        build_moe(nc, repeat=repeat)
        nc.compile()
        _CACHE[key] = nc
    return _CACHE[key]


def _pmajor(a2d, pdim_chunks):
    """[D_outer*P, X] -> [P, D_outer, X] partition-major contiguous."""
    d, x = a2d.shape
    return np.ascontiguousarray(
        a2d.reshape(pdim_chunks, P, x).transpose(1, 0, 2)
    )


def prep_in_maps(x, Wg, Wd, Wu):
    bf = ml_dtypes.bfloat16
    xf = np.asarray(x, np.float32).reshape(T, D)
    xTf = np.ascontiguousarray(xf.T)                       # [D, T]
    WgTh = _pmajor(
        np.ascontiguousarray(np.asarray(Wg, np.float32).T), NDC
    )                                                      # [P, NDC, E]
    WdT = np.asarray(Wd, np.float32).transpose(0, 2, 1)    # [E, D, DG]
    # pair p holds experts (2p, 2p+1) side by side on the free dim
    Wdp_c = np.concatenate([WdT[0::2], WdT[1::2]], axis=2).astype(bf)
    Wdp_h = np.ascontiguousarray(
        Wdp_c.reshape(NPAIR, NDC, P, 2 * DG).transpose(0, 2, 1, 3)
    )                                                      # [NPAIR, P, NDC, 2*DG]
    WuT_c = np.asarray(Wu, np.float32).transpose(0, 2, 1).astype(bf)  # [E, DG, D]
    WuT_h = np.ascontiguousarray(
        WuT_c.reshape(E, NGC, P, D).transpose(0, 2, 1, 3)
    )                                                      # [E, P, NGC, D]
    idb_h = np.eye(P, dtype=bf)
    idf_h = np.eye(P, dtype=np.float32)
    shared = dict(WgT=WgTh, Wdp=Wdp_h, WuTt=WuT_h, idb=idb_h, idf=idf_h)
    in_maps = []
    for c in range(NCORES):
        m = dict(shared)
        m["xT"] = _pmajor(
            np.ascontiguousarray(xTf[:, c * TC : (c + 1) * TC]), NDC
        )                                                  # [P, NDC, TC]
        in_maps.append(m)
    return in_maps


def kernel(x, Wg, Wd, Wu, k):
    assert int(k) == 2, f"kernel hardcodes top-2 routing, got k={k}"
    nc = get_nc()
    in_maps = prep_in_maps(x, Wg, Wd, Wu)
    res = run_bass_kernel_spmd(nc, in_maps, core_ids=list(range(NCORES)))
    outs = [np.asarray(res.results[c]["out"], dtype=np.float32) for c in range(NCORES)]
    return np.ascontiguousarray(
        np.concatenate(outs, axis=0).reshape(B, L, D), dtype=np.float32
    )


# revision 29
# speedup vs baseline: 1.0434x; 1.0245x over previous
"""MoE routing kernel for Trainium2 (8 NeuronCores).

Math (per reference):
  S = sigmoid(x @ Wg^T); top-2 gates G at indices I; w[t,e] = G if selected else 0
  down = sum_e w[:,e] * (x @ Wd[e]^T)          # [T, Dg]  (shared across experts)
  up   = sum_e w[:,e] * (down @ Wu[e]^T)       # [T, D]

Strategy: data-parallel over tokens — each of the 8 cores handles T/8 = 512
tokens and computes all 8 experts densely (top-2 applied via gate weights).
Host passes x^T / Wg^T / per-expert-transposed Wd, Wu so every on-chip matmul
has its contraction dim on partitions with zero large on-chip transposes.

Per-core dataflow (tokens token-major, 4 tiles of 128):
  gate:  ST[e,t] = Wg^T-chunks (lhsT) x xT-chunks (rhs, fp32 — bf16 scores
         would flip ~1.4% of top-2 selections and blow the error budget)
         accumulated in PSUM; PE-transpose 128-token slices; top-2 via two
         reduce_max passes; w = sigmoid(Z) * (Z >= second_max)
  down:  P_pair[t, 2*Dg] += xT-chunk (lhsT, bf16) x WdT-pair-chunk (rhs) over
         16 K-chunks; Wd pairs stream through 2 SBUF buffers (each pair dies
         after its matmuls); combine dacc = sum_e w_e * P_e on DVE
  z:     Z_e = w_e * dacc (ACT per-partition scale, bf16); PE-transpose to
         ZT_e[g, t]
  up:    U[t, dblk] += ZT_e-chunk (lhsT) x WuT-chunk (rhs) accumulated over
         (e, g-chunk) in PSUM; assemble bf16 rows; one 512 KiB DMA per token
         tile (out is bf16; host casts back to f32).
"""

import numpy as np
import ml_dtypes

import concourse.bass as bass
import concourse.mybir as mybir
import concourse.tile as tile
from concourse.bacc import Bacc
from concourse.bass_utils import run_bass_kernel_spmd

BF16 = mybir.dt.bfloat16
F32 = mybir.dt.float32
I32 = mybir.dt.int32
AF = mybir.ActivationFunctionType
ALU = mybir.AluOpType
AX = mybir.AxisListType

NCORES = 8
B, L, D, E, DG = 2, 2048, 2048, 8, 256
T = B * L            # 4096 tokens
TC = T // NCORES     # 512 tokens per core
P = 128
NDC = D // P         # 16 contraction chunks over D
NTT = TC // P        # 4 token tiles per core
DBLK = 512           # free-dim block for the up matmul
NDB = D // DBLK      # 4
NPAIR = E // 2       # 4 expert pairs (2 experts share one PSUM bank)
NGC = DG // P        # 2 contraction chunks over Dg



def build_moe(nc: bass.Bass, repeat: int = 1):
    # All inputs are host-prepacked partition-major so every DMA row is one
    # long contiguous run (few descriptors per transfer).
    xT = nc.dram_tensor("xT", [P, NDC, TC], F32, kind="ExternalInput")
    WgT = nc.dram_tensor("WgT", [P, NDC, E], F32, kind="ExternalInput")
    Wdp = nc.dram_tensor("Wdp", [NPAIR, P, NDC, 2 * DG], BF16, kind="ExternalInput")
    WuTt = nc.dram_tensor("WuTt", [2, P, 4, NGC, D], BF16, kind="ExternalInput")
    idb = nc.dram_tensor("idb", [P, P], BF16, kind="ExternalInput")
    idf = nc.dram_tensor("idf", [P, P], F32, kind="ExternalInput")
    out = nc.dram_tensor("out", [TC, D], BF16, kind="ExternalOutput")

    with tile.TileContext(nc) as tc:
        with (
            tc.tile_pool(name="res", bufs=1) as res,
            tc.tile_pool(name="stream", bufs=3) as stream,
            tc.tile_pool(name="small", bufs=2) as small,
            tc.tile_pool(name="ps", bufs=1, space="PSUM") as ps,
        ):
          # repeat>1 builds a timing NEFF that executes the whole kernel R
          # times back-to-back so fixed dispatch overhead cancels in
          # (t_R - t_1) / (R - 1).
          # constants load once; the repeat loop (timing NEFF) reuses them,
          # matching a single kernel() call where they also load once.
          ident_b = res.tile([P, P], BF16, tag="identb", name="ident_b")
          nc.sync.dma_start(ident_b[:], idb[:, :])
          ident_f = res.tile([E, E], F32, tag="identf", name="ident_f")
          nc.sync.dma_start(ident_f[:], idf[:E, :E])
          wg_sb = res.tile([P, NDC, E], F32, tag="wg", name="wg_sb")
          nc.sync.dma_start(wg_sb[:], WgT[:, :, :])

          # PE warmup (first rep only): trip the HAM activity window so the
          # gate matmuls run at 2.4 GHz; later reps stay warm back-to-back.
          wps = ps.tile([P, P], F32, tag="bank", bufs=6, name="warm_ps")
          for _w in range(24):
              nc.tensor.matmul(wps[:], ident_b[:], ident_b[:], start=True, stop=True)

          for _rep in range(repeat):
            xbf = res.tile([P, NDC, TC], BF16, tag="xbf", name="xbf")

            # ---------- stream x^T in 4 big chunks: cast to bf16 + gate matmul ----------
            # gate matmuls are M=8 (8 experts): pack 4 dc-chunks into the 4
            # column-groups of the PE array (tile_position) so they run
            # concurrently; each 32-strip accumulates 4 of the 16 chunks.
            st_ps = ps.tile([P, TC], F32, tag="bank", bufs=6, name="st_ps")
            XCH = 4          # dc-chunks per DMA = one concurrent group
            NXC = NDC // XCH
            for xc in range(NXC):
                xt = stream.tile([P, XCH, TC], F32, tag="xt", bufs=4, name=f"xt{xc}")
                nc.sync.dma_start(xt[:], xT[:, xc * XCH : (xc + 1) * XCH, :])
                for sub in range(XCH):
                    nc.scalar.copy(
                        xbf[:, xc * XCH + sub, :], xt[:, sub, :]
                    )
                for sub in range(XCH):
                    dc = xc * XCH + sub
                    nc.tensor.matmul(
                        st_ps[32 * sub : 32 * sub + E, :],
                        wg_sb[:, dc, :],
                        xt[:, sub, :],
                        start=(xc == 0),
                        stop=(xc == NXC - 1),
                        tile_position=(0, 32 * sub),
                    )

            # ---------- expert weight loads (overlap with compute) ----------
            # wd pairs stream through 2 buffers (each dies after its down
            # pair); wu stays resident (the sparse up phase consumes it
            # faster than it could stream)
            def load_wd(pr):
                t = stream.tile([P, NDC, 2 * DG], BF16, tag="wd", bufs=3, name=f"wd{pr}")
                nc.sync.dma_start(t[:], Wdp[pr])
                return t
            wd_sb = [load_wd(0), load_wd(1)]
            wu_sb = []
            for h in range(2):
                t = res.tile([P, 4, NGC, D], BF16, tag=f"wu{h}", name=f"wu{h}")
                nc.sync.dma_start(t[:], WuTt[h])
                wu_sb.append(t)

            # ---------- gate: transpose to token-major, top-2, weights ----------
            st_sb = res.tile([E, TC], F32, tag="stsb", name="st_sb")
            nc.vector.tensor_copy(st_sb[:], st_ps[0:E, :])
            for j in range(1, XCH):
                nc.vector.tensor_tensor(
                    st_sb[:], st_sb[:], st_ps[32 * j : 32 * j + E, :], ALU.add
                )
            w_tiles = []
            for tt in range(NTT):
                ztok = ps.tile([P, E], F32, tag="tr", bufs=2, name=f"ztok{tt}")
                nc.tensor.transpose(
                    ztok[:], st_sb[:, tt * P : (tt + 1) * P], ident_f[:]
                )
                m1 = small.tile([P, 1], F32, tag="m1", name=f"m1_{tt}")
                nc.vector.reduce_max(m1[:], ztok[:], axis=AX.X)
                # tmp = Z + (Z == m1) * -1e30  (mask out the max)
                tmp = small.tile([P, E], F32, tag="tmp", name=f"tmp{tt}")
                nc.vector.tensor_scalar(
                    tmp[:], ztok[:], m1[:], -1e30, ALU.is_equal, ALU.mult
                )
                nc.vector.tensor_tensor(tmp[:], tmp[:], ztok[:], ALU.add)
                m2 = small.tile([P, 1], F32, tag="m2", name=f"m2_{tt}")
                nc.vector.reduce_max(m2[:], tmp[:], axis=AX.X)
                g = small.tile([P, E], F32, tag="g", name=f"g{tt}")
                nc.scalar.activation(g[:], ztok[:], AF.Sigmoid)
                msk = small.tile([P, E], F32, tag="msk", name=f"msk{tt}")
                nc.vector.tensor_scalar(msk[:], ztok[:], m2[:], None, ALU.is_ge)
                w = res.tile([P, E], F32, tag=f"w{tt}", name=f"w{tt}")
                nc.vector.tensor_tensor(w[:], g[:], msk[:], ALU.mult)
                w_tiles.append(w)

            # diag tiles for the z^T matmuls: built on DVE right after the
            # gate so they never queue behind the down combines
            dg4s_all = []
            for tt in range(NTT):
                dg4s = []
                for eg in range(2):
                    dg4 = stream.tile([P, 4 * P], BF16, tag="diag", bufs=8, name=f"dg{tt}_{eg}")
                    for i in range(4):
                        e = 4 * eg + i
                        nc.vector.tensor_scalar(
                            dg4[:, i * P : (i + 1) * P], ident_b[:],
                            w_tiles[tt][:, e : e + 1], None, ALU.mult,
                        )
                    dg4s.append(dg4)
                dg4s_all.append(dg4s)

            # ---------- down (dense) ----------
            dacc_bf = res.tile([P, NTT, DG], BF16, tag="daccbf", name="dacc_bf")
            daccs = {}
            for pr in range(NPAIR):
                e0, e1 = 2 * pr, 2 * pr + 1
                if pr + 2 < NPAIR:
                    wd_sb.append(load_wd(pr + 2))
                for tt in range(NTT):
                    ts_ = slice(tt * P, (tt + 1) * P)
                    pt = ps.tile(
                        [P, 2 * DG], F32, tag="bank", bufs=6, name=f"pd{pr}_{tt}"
                    )
                    for dc in range(NDC):
                        nc.tensor.matmul(
                            pt[:],
                            xbf[:, dc, ts_],
                            wd_sb[pr][:, dc, :],
                            start=(dc == 0),
                            stop=(dc == NDC - 1),
                        )
                    if pr == 0:
                        dacc = stream.tile(
                            [P, DG], F32, tag="dacc", bufs=4, name=f"dacc{tt}"
                        )
                        daccs[tt] = dacc
                        nc.vector.tensor_scalar(
                            dacc[:], pt[:, 0:DG],
                            w_tiles[tt][:, e0 : e0 + 1], None, ALU.mult,
                        )
                    else:
                        dacc = daccs[tt]
                        nc.vector.scalar_tensor_tensor(
                            dacc[:], pt[:, 0:DG],
                            w_tiles[tt][:, e0 : e0 + 1], dacc[:],
                            ALU.mult, ALU.add,
                        )
                    nc.vector.scalar_tensor_tensor(
                        dacc_bf[:, tt, :] if pr == NPAIR - 1 else dacc[:],
                        pt[:, DG : 2 * DG],
                        w_tiles[tt][:, e1 : e1 + 1], dacc[:],
                        ALU.mult, ALU.add,
                    )

            # ---------- z^T via diagonal-weight matmuls ----------
            # zT_e[g, tok] = sum_k dacc_bf[k, g] * (w_e[k] * I[k, tok])
            # One N=512 matmul covers 4 experts (diags side by side); no PE
            # transposes, no ACT scales. All 16 zt matmuls run before the up
            # streams so the PE sees a single zt->up handoff bubble.
            zt_sb = res.tile([P, NGC, E, TC], BF16, tag="zt", name="zt_sb")
            for tt in range(NTT):
                ts_ = slice(tt * P, (tt + 1) * P)
                for gc in range(NGC):
                    for eg in range(2):
                        ztp = ps.tile([P, 4 * P], F32, tag="bank", bufs=6, name=f"zt{tt}_{gc}_{eg}")
                        nc.tensor.matmul(
                            ztp[:],
                            dacc_bf[:, tt, gc * P : (gc + 1) * P],
                            dg4s_all[tt][eg][:],
                            start=True,
                            stop=True,
                        )
                        nc.scalar.copy(
                            zt_sb[:, gc, 4 * eg : 4 * eg + 4, ts_], ztp[:]
                        )

            # ---------- up ----------
            for tt in range(NTT):
                ts_ = slice(tt * P, (tt + 1) * P)
                usb = stream.tile([P, D], BF16, tag="usb", bufs=2, name=f"usb{tt}")
                for db in range(NDB):
                    u = ps.tile([P, DBLK], F32, tag="bank", bufs=6, name=f"u{tt}_{db}")
                    kk = 0
                    for e in range(E):
                        for gc in range(NGC):
                            nc.tensor.matmul(
                                u[:],
                                zt_sb[:, gc, e, ts_],
                                wu_sb[e // 4][:, e % 4, gc, db * DBLK : (db + 1) * DBLK],
                                start=(kk == 0),
                                stop=(kk == E * NGC - 1),
                            )
                            kk += 1
                    nc.scalar.copy(usb[:, db * DBLK : (db + 1) * DBLK], u[:])
                nc.sync.dma_start(out[tt * P : (tt + 1) * P, :], usb[:])
    return nc


_CACHE = {}


def get_nc(repeat: int = 1) -> bass.Bass:
    key = ("nc", repeat)
    if key not in _CACHE:
        nc = Bacc()
        build_moe(nc, repeat=repeat)
        nc.compile()
        _CACHE[key] = nc
    return _CACHE[key]


def _pmajor(a2d, pdim_chunks):
    """[D_outer*P, X] -> [P, D_outer, X] partition-major contiguous."""
    d, x = a2d.shape
    return np.ascontiguousarray(
        a2d.reshape(pdim_chunks, P, x).transpose(1, 0, 2)
    )


def prep_in_maps(x, Wg, Wd, Wu):
    bf = ml_dtypes.bfloat16
    xf = np.asarray(x, np.float32).reshape(T, D)
    xTf = np.ascontiguousarray(xf.T)                       # [D, T]
    WgTh = _pmajor(
        np.ascontiguousarray(np.asarray(Wg, np.float32).T), NDC
    )                                                      # [P, NDC, E]
    WdT = np.asarray(Wd, np.float32).transpose(0, 2, 1)    # [E, D, DG]
    # pair p holds experts (2p, 2p+1) side by side on the free dim
    Wdp_c = np.concatenate([WdT[0::2], WdT[1::2]], axis=2).astype(bf)
    Wdp_h = np.ascontiguousarray(
        Wdp_c.reshape(NPAIR, NDC, P, 2 * DG).transpose(0, 2, 1, 3)
    )                                                      # [NPAIR, P, NDC, 2*DG]
    WuT_c = np.asarray(Wu, np.float32).transpose(0, 2, 1).astype(bf)  # [E, DG, D]
    WuT_h = np.ascontiguousarray(
        WuT_c.reshape(2, 4, NGC, P, D).transpose(0, 3, 1, 2, 4)
    )                                                      # [2, P, 4, NGC, D]
    idb_h = np.eye(P, dtype=bf)
    idf_h = np.eye(P, dtype=np.float32)
    shared = dict(WgT=WgTh, Wdp=Wdp_h, WuTt=WuT_h, idb=idb_h, idf=idf_h)
    in_maps = []
    for c in range(NCORES):
        m = dict(shared)
        m["xT"] = _pmajor(
            np.ascontiguousarray(xTf[:, c * TC : (c + 1) * TC]), NDC
        )                                                  # [P, NDC, TC]
        in_maps.append(m)
    return in_maps


def kernel(x, Wg, Wd, Wu, k):
    assert int(k) == 2, f"kernel hardcodes top-2 routing, got k={k}"
    nc = get_nc()
    in_maps = prep_in_maps(x, Wg, Wd, Wu)
    res = run_bass_kernel_spmd(nc, in_maps, core_ids=list(range(NCORES)))
    outs = [np.asarray(res.results[c]["out"], dtype=np.float32) for c in range(NCORES)]
    return np.ascontiguousarray(
        np.concatenate(outs, axis=0).reshape(B, L, D), dtype=np.float32
    )
